# revision 1
# baseline (speedup 1.0000x reference)
"""Multi-head attention (B=16, S=1024, D=768, H=12) on 8 TRN2 NeuronCores.

Strategy: pure data parallelism — batch 16 is split 2-per-core; weights are
replicated. Each core runs an identical Bass/Tile program on its own x shard,
so no collectives are needed. Host-side marshaling pre-transposes x and the
weights into the d-major layouts the PE array contracts over.

Per-core program (b in 0..1, head-pairs hp in 0..5):
  - v  = x @ W_v^T           natural [t, e] layout, stored head-interleaved
                             with a ones column -> PV lhsT [k, 64+1] per head
  - qT2/kT2 [128, S]         two heads stacked on partitions (d-major)
  - scoresT[k,q] = k q^T     row-packed per head via tile_position (K=64)
  - exp on ACT (scale=1/8) -> bf16 SBUF tile
  - PV: out[dh+1, q] += v_ext.T @ exp   (row 64 accumulates the softmax denom)
  - normalize: one [65,512] copy to SBUF per accumulator (frees PSUM fast),
    denom rows staged at partitions 0/32/64/96, ONE batched DVE reciprocal
    per unit, gpsimd partition_broadcast, DVE mult -> attn_outT [d, t] (bf16)
  - y = attn_outT.T @ W_out^T + b_out  (bf16 matmuls; bias as K=1 matmul)

QKV/scores matmuls run float32r (~1.5e-4); PV + projection run bf16.

The attention inner loop is ACT(exp)-paced (~1.3us per kt step). To keep the
in-order PE busy and the ACT stream gapless across batch boundaries, vgen of
the next batch and the projection of the previous batch are emitted as
"fill" work interleaved between attention pipeline steps.
"""
import ml_dtypes
import numpy as np
import concourse.bacc as bacc
import concourse.tile as tile
from concourse import mybir
from concourse.bass_utils import run_bass_kernel_spmd

FP32 = mybir.dt.float32
FP32R = mybir.dt.float32r
BF16 = mybir.dt.bfloat16
MMDT = FP32R                         # dtype for qkv-gen/scores matmuls
NPMM = np.float32
EXP = mybir.ActivationFunctionType.Exp

B, S, D, H = 2, 1024, 768, 12       # per-core batch of 2
HP = H // 2                          # head pairs
DT = D // 128                        # d tiles (6)
KT = S // 128                        # k tiles (8)
QC = S // 512                        # q chunks (2)
TT = S // 128                        # t tiles per batch (8)
N_CORES = 8

_CACHE = {}


def build_nc():
    nc = bacc.Bacc(trn_type="TRN2")
    xT = nc.dram_tensor("xT", [D, B * S], MMDT, kind="ExternalInput")
    wqkvT = nc.dram_tensor("wqkvT", [D, 3 * D], MMDT, kind="ExternalInput")
    woutT = nc.dram_tensor("woutT", [D, D], BF16, kind="ExternalInput")
    bout = nc.dram_tensor("bout", [1, D], BF16, kind="ExternalInput")
    ones_d = nc.dram_tensor("ones_d", [128, 128], MMDT, kind="ExternalInput")
    y = nc.dram_tensor("y", [B * S, D], FP32, kind="ExternalOutput")

    with tile.TileContext(nc) as tc:
        with (
            tc.tile_pool(name="wq", bufs=1) as p_wq,
            tc.tile_pool(name="wo", bufs=1) as p_wo,
            tc.tile_pool(name="cst", bufs=1) as p_cst,
            tc.tile_pool(name="xt", bufs=1) as p_xt,
            tc.tile_pool(name="vv", bufs=2) as p_v,
            tc.tile_pool(name="ao", bufs=2) as p_ao,
            tc.tile_pool(name="qk", bufs=3) as p_qk,
            tc.tile_pool(name="exp", bufs=3) as p_exp,
            tc.tile_pool(name="oc", bufs=6) as p_oc,
            tc.tile_pool(name="dn", bufs=2) as p_dn,
            tc.tile_pool(name="yy", bufs=2) as p_y,
            tc.tile_pool(name="rb", bufs=2) as p_rb,
            tc.tile_pool(name="r0", bufs=2) as p_r0,
            tc.tile_pool(name="sc", bufs=2, space="PSUM") as p_sc,
            tc.tile_pool(name="gen", bufs=2, space="PSUM") as p_gen,
            tc.tile_pool(name="oacc", bufs=2, space="PSUM") as p_oacc,
        ):
            wq = p_wq.tile([128, DT, 3 * D], MMDT)
            wo = p_wo.tile([128, DT, D], BF16)
            bo = p_cst.tile([1, D], BF16)
            ones = p_cst.tile([1, 128], MMDT)
            ones_bf = p_cst.tile([1, 128], BF16)
            nc.sync.dma_start(bo[:], bout[:])
            nc.sync.dma_start(ones[:], ones_d[0:1, :])
            nc.vector.tensor_copy(ones_bf[:], ones[:].bitcast(FP32))
            for j in range(DT):
                nc.sync.dma_start(
                    wq[:, j, 2 * D:3 * D],
                    wqkvT[128 * j:128 * (j + 1), 2 * D:3 * D],
                )
            for j in range(DT):  # head-pair 0 q/k columns first
                nc.sync.dma_start(
                    wq[:, j, 0:128], wqkvT[128 * j:128 * (j + 1), 0:128]
                )
                nc.sync.dma_start(
                    wq[:, j, D:D + 128], wqkvT[128 * j:128 * (j + 1), D:D + 128]
                )
            def load_wq_rest():
                for j in range(DT):
                    nc.sync.dma_start(
                        wq[:, j, 128:D], wqkvT[128 * j:128 * (j + 1), 128:D]
                    )
                    nc.sync.dma_start(
                        wq[:, j, D + 128:2 * D],
                        wqkvT[128 * j:128 * (j + 1), D + 128:2 * D],
                    )
                    nc.sync.dma_start(wo[:, j, :], woutT[128 * j:128 * (j + 1), :])

            xts, vs, aos = {}, {}, {}

            def load_xt(b):
                xt = p_xt.tile([128, DT, S], MMDT, tag="xt")
                for j in range(DT):
                    nc.sync.dma_start(
                        xt[:, j, :], xT[128 * j:128 * (j + 1), b * S:(b + 1) * S]
                    )
                xts[b] = xt

            def alloc_v(b):
                v = p_v.tile([128, KT, H, 65], BF16, tag="vv")
                nc.vector.memset(v[:, :, :, 64], 1.0)
                vs[b] = v

            def vgen_fills(b):
                """16 closures: one [128,512-or-256] psum group + copy each."""
                fills = []
                for tt in range(TT):
                    for h0, nh in ((0, 8), (8, 4)):
                        def f(tt=tt, h0=h0, nh=nh, b=b):
                            xt, v = xts[b], vs[b]
                            vp = p_gen.tile([128, 512], FP32, tag="gen")
                            cw = nh * 64
                            for j in range(DT):
                                nc.tensor.matmul(
                                    vp[:, 0:cw],
                                    xt[:, j, tt * 128:(tt + 1) * 128],
                                    wq[:, j,
                                       2 * D + h0 * 64:2 * D + h0 * 64 + cw],
                                    start=(j == 0), stop=(j == DT - 1),
                                )
                            nc.vector.tensor_copy(
                                v[:, tt, h0:h0 + nh, 0:64],
                                vp[:, 0:cw].rearrange("p (h c) -> p h c", h=nh),
                            )
                        fills.append(f)
                return fills

            def proj_fills(b):
                """16 closures: y(b) projection, one psum chunk-group each."""
                fills = []
                for tt in range(TT):
                    box = {}
                    for ci, (c0, cw) in enumerate(((0, 512), (512, 256))):
                        def f(tt=tt, ci=ci, c0=c0, cw=cw, b=b, box=box):
                            ao = aos[b]
                            if ci == 0:
                                ys = p_y.tile([128, D], FP32, tag="yy")
                                box["ys"] = ys
                            ys = box["ys"]
                            yp = p_gen.tile([128, 512], FP32, tag="gen")
                            nc.tensor.matmul(
                                yp[:, 0:cw], ones_bf[:], bo[:, c0:c0 + cw],
                                start=True, stop=False,
                            )
                            for j in range(DT):
                                nc.tensor.matmul(
                                    yp[:, 0:cw],
                                    ao[:, j, tt * 128:(tt + 1) * 128],
                                    wo[:, j, c0:c0 + cw],
                                    start=False, stop=(j == DT - 1),
                                )
                            nc.vector.tensor_copy(ys[:, c0:c0 + cw], yp[:, 0:cw])
                            if ci == 1:
                                nc.sync.dma_start(
                                    y[b * S + tt * 128:b * S + (tt + 1) * 128, :],
                                    ys[:],
                                )
                        fills.append(f)
                return fills

            def qk_gen(b, part, hp, qc):
                qp = p_gen.tile([128, 512], FP32, tag="gen")
                for j in range(DT):
                    nc.tensor.matmul(
                        qp[:, :],
                        wq[:, j, part * D + 128 * hp:part * D + 128 * (hp + 1)],
                        xts[b][:, j, qc * 512:(qc + 1) * 512],
                        start=(j == 0), stop=(j == DT - 1),
                    )
                return qp

            def unit(b, hp, fills, fill_start=0, fills_per_step=1,
                     early_fills=()):
                """qkgen + attention (fill work interleaved); returns the
                normalize closure so the caller can defer it past the next
                unit's q/k copies (keeps the slow DVE reciprocal off the
                unit-boundary critical path)."""
                v, ao = vs[b], aos[b]
                qkt = []
                for part in range(2):  # 0 = q, 1 = k
                    sq = p_qk.tile([128, S], MMDT, tag="qk")
                    for qc in range(QC):
                        qp = qk_gen(b, part, hp, qc)
                        nc.vector.tensor_copy(
                            sq[:, qc * 512:(qc + 1) * 512], qp[:, :]
                        )
                    qkt.append(sq)
                qT2, kT2 = qkt

                ocs, oaccs = {}, {}
                dn = p_dn.tile([128, 512], FP32, tag="dn")
                nc.vector.memset(dn[:], 1.0)

                def scores_exp(qc, kt):
                    sc = p_sc.tile([128, 1024], FP32, tag="sc")
                    nc.tensor.matmul(
                        sc[:, 0:512],
                        kT2[0:64, kt * 128:(kt + 1) * 128],
                        qT2[0:64, qc * 512:(qc + 1) * 512],
                        start=True, stop=True, tile_position=(0, 0),
                    )
                    nc.tensor.matmul(
                        sc[:, 512:1024],
                        kT2[64:128, kt * 128:(kt + 1) * 128],
                        qT2[64:128, qc * 512:(qc + 1) * 512],
                        start=True, stop=True, tile_position=(64, 0),
                    )
                    ex = p_exp.tile([128, 1024], BF16, tag="exp")
                    nc.scalar.activation(ex[:], sc[:], EXP, scale=0.125)
                    return ex

                def pv(qc, kt, ex):
                    if kt == 0:
                        o_a = p_oacc.tile([65, 512], FP32, tag="oacc")
                        o_b = p_oacc.tile([65, 512], FP32, tag="oacc")
                        oaccs[(qc, 0)] = o_a
                        oaccs[(qc, 1)] = o_b
                    nc.tensor.matmul(
                        oaccs[(qc, 0)][:], v[:, kt, 2 * hp, :], ex[:, 0:512],
                        start=(kt == 0), stop=(kt == KT - 1),
                    )
                    nc.tensor.matmul(
                        oaccs[(qc, 1)][:], v[:, kt, 2 * hp + 1, :],
                        ex[:, 512:1024],
                        start=(kt == 0), stop=(kt == KT - 1),
                    )
                    if kt == KT - 1:
                        # one copy frees the PSUM accumulator; stage the
                        # denom row for the batched per-unit reciprocal
                        for head in range(2):
                            oc = p_oc.tile([65, 512], FP32, tag="oc")
                            nc.vector.tensor_copy(oc[:], oaccs[(qc, head)][:])
                            r_idx = 32 * (2 * qc + head)
                            nc.vector.tensor_copy(
                                dn[r_idx:r_idx + 1, :], oc[64:65, :]
                            )
                            ocs[(qc, head)] = oc

                # 16-step pipeline: scores/exp one step ahead of PV; fill
                # work drips in between steps
                eq = list(early_fills)
                fq = list(fills)
                prev = None
                step = 0
                for qc in range(QC):
                    for kt in range(KT):
                        ex = scores_exp(qc, kt)
                        if prev is not None:
                            pv(*prev)
                        prev = (qc, kt, ex)
                        if eq:
                            eq.pop(0)()
                        if step >= fill_start:
                            for _ in range(fills_per_step):
                                if fq:
                                    fq.pop(0)()
                        step += 1
                pv(*prev)
                for f in eq:
                    f()
                for f in fq:
                    f()

                def normalize():
                    # batched reciprocal of the unit's 4 denom rows
                    dnr = p_dn.tile([128, 512], FP32, tag="dnr")
                    nc.vector.reciprocal(dnr[:], dn[:])
                    for qc in range(QC):
                        for head in range(2):
                            r_idx = 32 * (2 * qc + head)
                            # partition_broadcast only honors base-partition-0
                            # inputs on HW; shift the row down first
                            r0 = p_r0.tile([1, 512], FP32, tag="r0")
                            nc.vector.tensor_copy(
                                r0[:], dnr[r_idx:r_idx + 1, :])
                            rb = p_rb.tile([64, 512], FP32, tag="rb")
                            nc.gpsimd.partition_broadcast(rb[:], r0[:])
                            nc.vector.tensor_mul(
                                ao[64 * head:64 * (head + 1), hp,
                                   qc * 512:(qc + 1) * 512],
                                ocs[(qc, head)][0:64, :], rb[:],
                            )
                return normalize

            # ---- schedule ----
            load_xt(0)
            alloc_v(0)
            load_wq_rest()
            norm = None
            for b in range(B):
                ao = p_ao.tile([128, DT, S], BF16, tag="ao")
                aos[b] = ao
                for hp in range(HP):
                    early = [norm] if norm is not None else []
                    if hp == HP - 1 and b + 1 < B:
                        # stage next batch's x and v; vgen fills into this
                        # unit's attention (later steps — the xT DMA needs a
                        # few steps of headroom)
                        load_xt(b + 1)
                        alloc_v(b + 1)
                        norm = unit(b, hp, vgen_fills(b + 1),
                                    fill_start=7, fills_per_step=2,
                                    early_fills=early)
                    elif hp == 0 and b == 0:
                        # this batch's own vgen fills the first unit; the
                        # 2-per-step pace keeps each v[tt] emitted just ahead
                        # of the PV that consumes it (in-order PE guarantees
                        # execution order)
                        norm = unit(b, hp, vgen_fills(0), fills_per_step=2,
                                    early_fills=early)
                    elif hp == 0 and b > 0:
                        # previous batch's projection fills this unit; the
                        # deferred normalize of unit (b-1, 5) must land
                        # before any projection group reads its ao band
                        norm = unit(b, hp, proj_fills(b - 1),
                                    fill_start=1, early_fills=early)
                    else:
                        norm = unit(b, hp, [], early_fills=early)
            norm()
            for f in proj_fills(B - 1):
                f()
    nc.finalize()
    return nc


def _marshal(x, W_qkv, W_out, b_out):
    wqkvT = np.ascontiguousarray(W_qkv.T).astype(NPMM)
    woutT = np.ascontiguousarray(W_out.T).astype(ml_dtypes.bfloat16)
    bo = np.ascontiguousarray(b_out.reshape(1, D)).astype(ml_dtypes.bfloat16)
    ones = np.ones((128, 128), dtype=NPMM)
    in_maps = []
    for c in range(N_CORES):
        xc = np.ascontiguousarray(
            np.asarray(x)[B * c:B * (c + 1)].reshape(B * S, D).T
        ).astype(NPMM)
        in_maps.append({
            "xT": xc, "wqkvT": wqkvT, "woutT": woutT, "bout": bo,
            "ones_d": ones,
        })
    return in_maps


def run(x, W_qkv, W_out, b_out, trace=False, **spmd_kwargs):
    if "nc" not in _CACHE:
        _CACHE["nc"] = build_nc()
    nc = _CACHE["nc"]
    in_maps = _marshal(x, W_qkv, W_out, b_out)
    res = run_bass_kernel_spmd(
        nc, in_maps, core_ids=list(range(N_CORES)), trace=trace, **spmd_kwargs
    )
    out = np.stack([res.results[c]["y"] for c in range(N_CORES)], axis=0)
    out = out.reshape(N_CORES * B, S, D)
    return out, res


def kernel(x, W_qkv, W_out, b_out):
    out, _ = run(x, W_qkv, W_out, b_out)
    return out



# revision 6
# speedup vs baseline: 1.1887x; 1.1887x over previous
"""Multi-head attention (B=16, S=1024, D=768, H=12) on 8 TRN2 NeuronCores.

Strategy: pure data parallelism — batch 16 is split 2-per-core; weights are
replicated. Each core runs an identical Bass/Tile program on its own x shard,
so no collectives are needed. Host-side marshaling pre-transposes x and the
weights into the d-major layouts the PE array contracts over.

All matmul inputs are bf16 (fp32 PSUM accumulation). Per-core program
(b in 0..1, head pairs hp in 0..5, unit u = 6b+hp):

  - qT2/kT2 [128, S]  two heads stacked on partitions (d-major, bf16),
    generated by matmul groups emitted as FILL work inside the previous
    unit's attention steps (no serial region at unit boundaries)
  - per step (qc, kt): two scores matmuls (one per head via tile_position
    row packing) into separate [128,512] PSUM tiles, each followed by its
    own ACT exp (scale=1/8) -> bf16 SBUF tile. Splitting per-head halves
    the producer->consumer latency the PE must cover.
  - PV: out[dh+1, q] += v_ext.T @ exp (row 64 accumulates the denominator
    via a ones column in v); PV for step i is emitted AFTER step i+1's
    scores and the step's fill work, so the exp dependency is always
    satisfied when the in-order PE reaches it -> no per-step stall, the
    PE stays continuously busy and ramps to its max pstate.
  - fills: qkgen for unit u+1 (every unit), vgen (units 0-1 for batch 0,
    units 3-5 for batch 1), projection of batch b-1 (units 6-8).
  - normalize: denom rows staged compactly at partitions 0-3, one
    reciprocal_approx_fast per unit, gpsimd partition_broadcast, DVE mult
    -> attn_outT [d, t] (bf16); deferred into the next unit.
  - y = attn_outT.T @ W_out^T + b_out; bias added by the DVE during the
    PSUM->SBUF move (tensor_add with a host-prebroadcast [128, D] bias),
    not by K=1 matmuls.
"""
import ml_dtypes
import numpy as np
import concourse.bacc as bacc
import concourse.tile as tile
from concourse import mybir
from concourse.bass_utils import run_bass_kernel_spmd

FP32 = mybir.dt.float32
BF16 = mybir.dt.bfloat16
EXP = mybir.ActivationFunctionType.Exp

B, S, D, H = 2, 1024, 768, 12       # per-core batch of 2
HP = H // 2                          # head pairs (6)
DT = D // 128                        # d tiles (6)
KT = S // 128                        # k tiles (8)
QC = S // 512                        # q chunks (2)
TT = S // 128                        # t tiles per batch (8)
NU = B * HP                          # units (12)
N_CORES = 8

_CACHE = {}


def build_nc():
    nc = bacc.Bacc(trn_type="TRN2")
    xT = nc.dram_tensor("xT", [D, B * S], BF16, kind="ExternalInput")
    wqkvT = nc.dram_tensor("wqkvT", [D, 3 * D], BF16, kind="ExternalInput")
    woutT = nc.dram_tensor("woutT", [D, D], BF16, kind="ExternalInput")
    bbc = nc.dram_tensor("bbc", [128, D], BF16, kind="ExternalInput")
    y = nc.dram_tensor("y", [B * S, D], FP32, kind="ExternalOutput")

    with tile.TileContext(nc) as tc:
        with (
            tc.tile_pool(name="wq", bufs=1) as p_wq,
            tc.tile_pool(name="wo", bufs=1) as p_wo,
            tc.tile_pool(name="cst", bufs=1) as p_cst,
            tc.tile_pool(name="xt", bufs=2) as p_xt,
            tc.tile_pool(name="vv", bufs=2) as p_v,
            tc.tile_pool(name="ao", bufs=2) as p_ao,
            tc.tile_pool(name="qk", bufs=4) as p_qk,
            tc.tile_pool(name="exp", bufs=4) as p_exp,
            tc.tile_pool(name="oc", bufs=6) as p_oc,
            tc.tile_pool(name="yy", bufs=2) as p_y,
            tc.tile_pool(name="rb", bufs=2) as p_rb,
            tc.tile_pool(name="r0", bufs=2) as p_r0,
            tc.tile_pool(name="sc", bufs=4, space="PSUM") as p_sc,
            tc.tile_pool(name="gen", bufs=2, space="PSUM") as p_gen,
            tc.tile_pool(name="oacc", bufs=2, space="PSUM") as p_oacc,
        ):
            wq = p_wq.tile([128, DT, 3 * D], BF16)
            wo = p_wo.tile([128, DT, D], BF16)
            bb = p_cst.tile([128, D], BF16)

            xts, vs, aos = {}, {}, {}

            # ---- DMA priority order ----
            # 1) unit-0 q/k weight columns (needed by the prologue qkgen)
            for j in range(DT):
                nc.sync.dma_start(
                    wq[:, j, 0:128], wqkvT[128 * j:128 * (j + 1), 0:128]
                )
                nc.sync.dma_start(
                    wq[:, j, D:D + 128], wqkvT[128 * j:128 * (j + 1), D:D + 128]
                )

            def load_xt(b):
                xt = p_xt.tile([128, DT, S], BF16, tag="xt")
                for qc in range(QC):
                    for j in range(DT):
                        nc.sync.dma_start(
                            xt[:, j, qc * 512:(qc + 1) * 512],
                            xT[128 * j:128 * (j + 1),
                               b * S + qc * 512:b * S + (qc + 1) * 512],
                        )
                xts[b] = xt

            load_xt(0)  # 2) x shard, qc0 chunks first

            for j in range(DT):  # 3) v weight columns (vgen fills, unit 0)
                nc.sync.dma_start(
                    wq[:, j, 2 * D:3 * D],
                    wqkvT[128 * j:128 * (j + 1), 2 * D:3 * D],
                )

            def load_rest():  # 4) remaining q/k cols, w_out, bias
                for j in range(DT):
                    nc.sync.dma_start(
                        wq[:, j, 128:D], wqkvT[128 * j:128 * (j + 1), 128:D]
                    )
                    nc.sync.dma_start(
                        wq[:, j, D + 128:2 * D],
                        wqkvT[128 * j:128 * (j + 1), D + 128:2 * D],
                    )
                    nc.sync.dma_start(wo[:, j, :], woutT[128 * j:128 * (j + 1), :])
                nc.sync.dma_start(bb[:], bbc[:])

            def alloc_v(b):
                v = p_v.tile([128, KT, H, 65], BF16, tag="vv")
                nc.vector.memset(v[:, :, :, 64], 1.0)
                vs[b] = v

            # ---- q/k generation (d-major bf16), per unit, as fill work ----
            qks = {}

            def ensure_qk(u):
                if u not in qks:
                    qks[u] = (
                        p_qk.tile([128, S], BF16, tag="qk", name=f"q{u}"),
                        p_qk.tile([128, S], BF16, tag="qk", name=f"k{u}"),
                    )
                return qks[u]

            def qkgen_group(u, part, qc):
                b2, hp2 = divmod(u, HP)

                def f():
                    tq = ensure_qk(u)[part]
                    qp = p_gen.tile([128, 512], FP32, tag="gen")
                    for j in range(DT):
                        nc.tensor.matmul(
                            qp[:],
                            wq[:, j,
                               part * D + 128 * hp2:part * D + 128 * (hp2 + 1)],
                            xts[b2][:, j, qc * 512:(qc + 1) * 512],
                            start=(j == 0), stop=(j == DT - 1),
                        )
                    nc.vector.tensor_copy(tq[:, qc * 512:(qc + 1) * 512], qp[:])
                return f

            def qkgen_fills(u):
                return [
                    qkgen_group(u, 0, 0), qkgen_group(u, 1, 0),
                    qkgen_group(u, 1, 1), qkgen_group(u, 0, 1),
                ]

            # ---- v generation fills: one closure per (tt, head group) ----
            def vgen_fills(b, h0, nh):
                fills = []
                for tt in range(TT):
                    def f(tt=tt, h0=h0, nh=nh, b=b):
                        xt, v = xts[b], vs[b]
                        vp = p_gen.tile([128, 512], FP32, tag="gen")
                        cw = nh * 64
                        for j in range(DT):
                            nc.tensor.matmul(
                                vp[:, 0:cw],
                                xt[:, j, tt * 128:(tt + 1) * 128],
                                wq[:, j, 2 * D + h0 * 64:2 * D + h0 * 64 + cw],
                                start=(j == 0), stop=(j == DT - 1),
                            )
                        nc.vector.tensor_copy(
                            v[:, tt, h0:h0 + nh, 0:64],
                            vp[:, 0:cw].rearrange("p (h c) -> p h c", h=nh),
                        )
                    fills.append(f)
                return fills

            # ---- output projection fills (bias via DVE tensor_add) ----
            def proj_fills(b):
                fills = []
                for tt in range(TT):
                    box = {}
                    for ci, (c0, cw) in enumerate(((0, 512), (512, 256))):
                        def f(tt=tt, ci=ci, c0=c0, cw=cw, b=b, box=box):
                            ao = aos[b]
                            if ci == 0:
                                box["ys"] = p_y.tile([128, D], FP32, tag="yy", name=f"ys{b}_{tt}")
                            ys = box["ys"]
                            yp = p_gen.tile([128, 512], FP32, tag="gen")
                            for j in range(DT):
                                nc.tensor.matmul(
                                    yp[:, 0:cw],
                                    ao[:, j, tt * 128:(tt + 1) * 128],
                                    wo[:, j, c0:c0 + cw],
                                    start=(j == 0), stop=(j == DT - 1),
                                )
                            nc.vector.tensor_add(
                                ys[:, c0:c0 + cw], yp[:, 0:cw], bb[:, c0:c0 + cw]
                            )
                            if ci == 1:
                                nc.sync.dma_start(
                                    y[b * S + tt * 128:b * S + (tt + 1) * 128, :],
                                    ys[:],
                                )
                        fills.append(f)
                return fills

            # ---- one attention unit (16 steps, fills interleaved) ----
            def unit(u, fills, pace=None):
                b, hp = divmod(u, HP)
                v, ao = vs[b], aos[b]
                qT2, kT2 = ensure_qk(u)
                ocs, oaccs = {}, {}

                def scores_exp(qc, kt, half):
                    sc = p_sc.tile([128, 512], FP32, tag="sc")
                    nc.tensor.matmul(
                        sc[:],
                        kT2[64 * half:64 * (half + 1), kt * 128:(kt + 1) * 128],
                        qT2[64 * half:64 * (half + 1), qc * 512:(qc + 1) * 512],
                        start=True, stop=True, tile_position=(64 * half, 0),
                    )
                    ex = p_exp.tile([128, 512], BF16, tag="exp")
                    nc.scalar.activation(ex[:], sc[:], EXP, scale=0.125)
                    return ex

                def pv(qc, kt, half, ex):
                    key = (qc, half)
                    if kt == 0:
                        oaccs[key] = p_oacc.tile([65, 512], FP32, tag="oacc", name=f"oacc{key[0]}_{key[1]}")
                    nc.tensor.matmul(
                        oaccs[key][:], v[:, kt, 2 * hp + half, :], ex[:],
                        start=(kt == 0), stop=(kt == KT - 1),
                    )
                    if kt == KT - 1:
                        # one copy frees the PSUM accumulator
                        oc = p_oc.tile([65, 512], FP32, tag="oc")
                        nc.vector.tensor_copy(oc[:], oaccs[key][:])
                        ocs[key] = oc

                fq = list(fills)
                steps = [(qc, kt) for qc in range(QC) for kt in range(KT)]
                prev = None
                for i, (qc, kt) in enumerate(steps):
                    exA = scores_exp(qc, kt, 0)
                    exB = scores_exp(qc, kt, 1)
                    n = pace if pace is not None else -(-len(fq) // (len(steps) - i))
                    for _ in range(min(n, len(fq))):
                        fq.pop(0)()
                    if prev is not None:
                        pv(*prev[0])
                        pv(*prev[1])
                    prev = ((qc, kt, 0, exA), (qc, kt, 1, exB))
                pv(*prev[0])
                pv(*prev[1])
                for f in fq:
                    f()

                def normalize():
                    for qc in range(QC):
                        for half in range(2):
                            # partition_broadcast only honors base-partition-0
                            # inputs on HW; the reciprocal shifts the denom
                            # row 64 -> 0 (both are legal quadrant bases)
                            r0 = p_r0.tile([1, 512], FP32, tag="r0")
                            nc.vector.reciprocal_approx_fast(
                                r0[:], ocs[(qc, half)][64:65, :])
                            rb = p_rb.tile([64, 512], FP32, tag="rb")
                            nc.gpsimd.partition_broadcast(rb[:], r0[:])
                            nc.vector.tensor_mul(
                                ao[64 * half:64 * (half + 1), hp,
                                   qc * 512:(qc + 1) * 512],
                                ocs[(qc, half)][0:64, :], rb[:],
                            )
                return normalize

            # ---- schedule ----
            alloc_v(0)
            load_rest()
            # prologue: q/k (qc0 halves) for unit 0; the qc1 halves and all
            # later units' q/k generation run as fill work
            ensure_qk(0)
            qkgen_group(0, 0, 0)()
            qkgen_group(0, 1, 0)()
            u0_late = [qkgen_group(0, 1, 1), qkgen_group(0, 0, 1)]

            vg1 = None
            norm = None
            for u in range(NU):
                b, hp = divmod(u, HP)
                if hp == 0:
                    aos[b] = p_ao.tile([128, DT, S], BF16, tag="ao", name=f"ao{b}")
                fills = [norm] if norm is not None else []
                pace = None
                if u == 0:
                    # k-qc1 needed by step 4, q-qc1 by step 8, vgen(tt) by
                    # step tt+1; pace=2 front-loads to meet all deadlines
                    vg = vgen_fills(0, 0, 8)
                    fills += [u0_late[0], vg[0], u0_late[1]] + vg[1:]
                    fills += qkgen_fills(1)
                    pace = 2
                else:
                    if u + 1 < NU:
                        fills += qkgen_fills(u + 1)
                    if u == 1:
                        fills += vgen_fills(0, 8, 4)
                        load_xt(1)  # batch-1 x: ~2 units of DMA headroom
                    if u == 2:
                        alloc_v(1)
                        vg1 = vgen_fills(1, 0, 8) + vgen_fills(1, 8, 4)
                    if b == 0 and hp == 3:
                        fills += vg1[0:6]
                    if b == 0 and hp == 4:
                        fills += vg1[6:11]
                    if b == 0 and hp == 5:
                        fills += vg1[11:16]
                    if b == 1 and hp == 0:
                        pr0 = proj_fills(0)
                        fills += pr0[0:6]
                    if b == 1 and hp == 1:
                        fills += pr0[6:11]
                    if b == 1 and hp == 2:
                        fills += pr0[11:16]
                norm = unit(u, fills, pace=pace)
            norm()
            for f in proj_fills(B - 1):
                f()
    nc.finalize()
    return nc


def _marshal(x, W_qkv, W_out, b_out):
    wqkvT = np.ascontiguousarray(W_qkv.T).astype(ml_dtypes.bfloat16)
    woutT = np.ascontiguousarray(W_out.T).astype(ml_dtypes.bfloat16)
    bbc = np.ascontiguousarray(
        np.broadcast_to(np.asarray(b_out).reshape(1, D), (128, D))
    ).astype(ml_dtypes.bfloat16)
    in_maps = []
    for c in range(N_CORES):
        xc = np.ascontiguousarray(
            np.asarray(x)[B * c:B * (c + 1)].reshape(B * S, D).T
        ).astype(ml_dtypes.bfloat16)
        in_maps.append({"xT": xc, "wqkvT": wqkvT, "woutT": woutT, "bbc": bbc})
    return in_maps


def run(x, W_qkv, W_out, b_out, trace=False, **spmd_kwargs):
    if "nc" not in _CACHE:
        _CACHE["nc"] = build_nc()
    nc = _CACHE["nc"]
    in_maps = _marshal(x, W_qkv, W_out, b_out)
    res = run_bass_kernel_spmd(
        nc, in_maps, core_ids=list(range(N_CORES)), trace=trace, **spmd_kwargs
    )
    out = np.stack([res.results[c]["y"] for c in range(N_CORES)], axis=0)
    out = out.reshape(N_CORES * B, S, D)
    return out, res


def kernel(x, W_qkv, W_out, b_out):
    out, _ = run(x, W_qkv, W_out, b_out)
    return out
